# revision 14
# baseline (speedup 1.0000x reference)
"""CACombiner Trainium2 kernel: conv-projected efficient attention + FFN.

Data-parallel over batch: 8 batch elements -> 8 NeuronCores, identical SPMD
program per core.

Key tricks:
  - q/k/v projections as fp8e4m3 DoubleRow matmuls (K=256/instr).
  - Wr folded into the normalized context: WrCT = ctx_bd^T @ Wr^T computed
    once on-device, so reprojection is a single fp8 DoubleRow GEMM of the
    softmaxed q (stored fp8 x64) -- no att intermediate at all.
  - ELU via elu(x)+1 = max(x+1, min(e^x, 1)): the +1 rides the FFN1 bias
    fold, and the resulting he+1 offset is corrected by subtracting
    rowsum(W2) at the FFN2 eviction.
  - FFN in bf16 (fp8 fails the accuracy budget there); LN stats via
    ones-vector matmuls on bf16 copies.

Structure per core:
  phase 1  (16 x 256-l pairs): q softmax -> qsm8, exp(k), v, ctx/S accum
  phase 2a (8 x 512-l tiles):  fused reprojection + residual, LN1 -> zr
  phase 2b (8 x 512-l tiles):  FFN1 + ELU + FFN2, LN2 -> out
"""
import sys
sys.path.insert(0, "/opt/trn_rl_repo")
from contextlib import ExitStack

import numpy as np

import concourse.bass as bass
import concourse.tile as tile
from concourse import mybir, bacc
from concourse.bass_utils import run_bass_kernel_spmd
from concourse.alu_op_type import AluOpType

F32 = mybir.dt.float32
F32R = mybir.dt.float32r
BF16 = mybir.dt.bfloat16
F8 = mybir.dt.float8e4
AFT = mybir.ActivationFunctionType
Ax = mybir.AxisListType
DR = mybir.MatmulPerfMode.DoubleRow

B, C, L = 8, 512, 4096
H, DK = 8, 64
EPS = 1e-5
CC = C // 128           # 4 channel chunks
NP1 = L // 256          # 16 phase-1 pair-tiles (2x128 l)
NL2 = L // 512          # 8 phase-2 l-tiles

SW = 32.0               # weight scale for fp8
SA = 256.0              # ctx scale for bf16/fp8
SQ = 64.0               # qsm scale for fp8
ZDESC = 1.0 / (SA * SQ)  # descale for fused reprojection output
LN64 = float(np.log(64.0))

_CACHE = {}
LAST_RESULT = None


def _build_program(gates):
    (HAS_BQ, HAS_BK, HAS_BV, HAS_BR, HAS_B2, HAS_G2, HAS_BE2) = gates
    nc = bacc.Bacc("TRN2", target_bir_lowering=False, debug=False)

    def din(name, shape, dtype):
        return nc.dram_tensor(name, list(shape), dtype, kind="ExternalInput").ap()

    z1d = din("z1", (C, L), F32)
    z2d = din("z2", (C, L), F32)
    Wq8_d = din("Wq8", (128, CC, 512), F8)
    Wkv8_d = din("Wkv8", (128, CC, 1024), F8)
    WrTb_d = din("WrTb", (128, CC, 512), BF16)
    W1gb_d = din("W1gb", (128, CC, 1024), BF16)
    W2gb_d = din("W2gb", (128, 8, 512), BF16)
    U1f_d = din("U1f", (1, 2, 1024), F8)
    onesrow_d = din("onesrow", (1, 2, NL2, 512), F8)
    nw2s_c_d = din("nw2s_c", (128, CC), F32)
    inv512b_d = din("inv512b", (128, 1), BF16)
    ones1x128_d = din("ones1x128", (1, 128), F32R)
    identf8_d = din("identf8", (128, 128), F8)
    identb_d = din("identb", (128, 128), BF16)
    ones_f8_d = din("ones_f8", (128, 2, 1), F8)
    eps11_d = din("eps11", (1, 1), F32)
    negone_d = din("negone", (128, 1), F32)
    # gated bias constants (all-zero in the common case)
    bq32_d = din("bq32", (1, 512), F32R)
    bk32_d = din("bk32", (1, 512), F32R)
    bv_c_d = din("bv_c", (128, CC), F32)
    br_c_d = din("br_c", (128, CC), F32)
    g2_c_d = din("g2_c", (128, CC), F32)
    be2_c_d = din("be2_c", (128, CC), F32)
    outd = nc.dram_tensor("out", [C, L], F32, kind="ExternalOutput").ap()

    z1r = z1d.rearrange("(cc p) l -> p cc l", p=128)
    z2r = z2d.rearrange("(cc p) l -> p cc l", p=128)
    outr = outd.rearrange("(cc p) l -> p cc l", p=128)

    mm = nc.tensor.matmul
    tt = nc.vector.tensor_tensor
    ts = nc.vector.tensor_scalar
    stt = nc.vector.scalar_tensor_tensor
    act = nc.scalar.activation
    pts = nc.gpsimd.tensor_scalar
    pstt = nc.gpsimd.scalar_tensor_tensor
    ptt = nc.gpsimd.tensor_tensor
    pcopy = nc.gpsimd.tensor_copy

    with tile.TileContext(nc) as tc, ExitStack() as ctx:
        cpool = ctx.enter_context(tc.tile_pool(name="consts", bufs=1))

        def const_tile(shape, dtype, src, tag, defer=False):
            t = cpool.tile(list(shape), dtype, tag=tag, name=tag)
            if defer:
                deferred_dmas.append((t, src))
            else:
                nc.sync.dma_start(t[:], src)
            return t

        deferred_dmas = []
        # loaded up front: everything phase 1 touches
        identf8 = const_tile((128, 128), F8, identf8_d, "identf8")
        ones_f8 = const_tile((128, 2, 1), F8, ones_f8_d, "ones_f8")
        Wq8 = const_tile((128, CC, 512), F8, Wq8_d, "Wq8")
        Wkv8 = const_tile((128, CC, 1024), F8, Wkv8_d, "Wkv8")
        inv512b = const_tile((128, 1), BF16, inv512b_d, "inv512b")
        ones1x128 = const_tile((1, 128), F32R, ones1x128_d, "ones1x128")
        identb = const_tile((128, 128), BF16, identb_d, "identb")
        eps11 = const_tile((1, 1), F32, eps11_d, "eps11")
        negone = const_tile((128, 1), F32, negone_d, "negone")
        # loaded during phase 1 (consumed by finalize / phase 2)
        WrTb = const_tile((128, CC, 512), BF16, WrTb_d, "WrTb", defer=True)
        W1gb = const_tile((128, CC, 1024), BF16, W1gb_d, "W1gb", defer=True)
        W2gb = const_tile((128, 8, 512), BF16, W2gb_d, "W2gb", defer=True)
        U1f = const_tile((1, 2, 1024), F8, U1f_d, "U1f", defer=True)
        nw2s_c = const_tile((128, CC), F32, nw2s_c_d, "nw2s_c", defer=True)
        if HAS_BQ:
            bq32 = const_tile((1, 512), F32R, bq32_d, "bq32")
        if HAS_BK:
            bk32 = const_tile((1, 512), F32R, bk32_d, "bk32")
        if HAS_BV:
            bv_c = const_tile((128, CC), F32, bv_c_d, "bv_c")
        if HAS_BR:
            br_c = const_tile((128, CC), F32, br_c_d, "br_c")
        if HAS_G2:
            g2_c = const_tile((128, CC), F32, g2_c_d, "g2_c")
        if HAS_BE2:
            be2_c = const_tile((128, CC), F32, be2_c_d, "be2_c")

        # persistent across phases
        qsm8 = cpool.tile([128, CC, L], F8, tag="qsm8", name="qsm8")
        WrCT8 = cpool.tile([128, CC, 512], F8, tag="WrCT8", name="WrCT8")
        zr_all = cpool.tile([128, NL2, CC, 512], BF16, tag="zr", name="zr_all")
        mur_all = cpool.tile([1, 2, NL2, 512], F8, tag="mur", name="mur_all")
        deferred_dmas.append((mur_all[:], onesrow_d))

        # ---------------- Phase 1: q softmax + k/v + ctx accumulation --------
        with ExitStack() as p1:
            lp = p1.enter_context(tc.tile_pool(name="lp1", bufs=2))
            psw = p1.enter_context(tc.tile_pool(name="psw", bufs=2, space="PSUM"))
            pst = p1.enter_context(tc.tile_pool(name="pst", bufs=1, space="PSUM"))
            psc = p1.enter_context(tc.tile_pool(name="psc", bufs=1, space="PSUM"))

            ctxps = psc.tile([128, CC, 128], F32, tag="ctxps", name="ctxps")
            Sps = psc.tile([128, CC], F32, tag="Sps", name="Sps")

            for p in range(NP1):
                l0 = p * 256
                sl = slice(l0, l0 + 256)
                z1c = lp.tile([128, CC, 256], F32, tag="z1c")
                nc.sync.dma_start(z1c[:], z1r[:, :, sl])
                z2c = lp.tile([128, CC, 256], F32, tag="z2c")
                nc.sync.dma_start(z2c[:], z2r[:, :, sl])
                z1f8 = lp.tile([128, CC, 256], F8, tag="z1f8")
                pcopy(z1f8[:], z1c[:])
                z2f8 = lp.tile([128, CC, 256], F8, tag="z2f8")
                pcopy(z2f8[:], z2c[:])

                # qT [l,o] fp8 DoubleRow (values = SW * q_true)
                qps = psw.tile([128, 2, 512], F32, tag="pw", name="qps")
                for i in range(2):
                    ls = slice(i * 128, (i + 1) * 128)
                    mm(qps[:, i, :], z1f8[:, 0:2, ls], Wq8[:, 0:2, :],
                       start=True, stop=False, perf_mode=DR)
                    mm(qps[:, i, :], z1f8[:, 2:4, ls], Wq8[:, 2:4, :],
                       start=False, stop=not HAS_BQ, perf_mode=DR)
                    if HAS_BQ:
                        mm(qps[:, i, :], ones1x128[:], bq32[:],
                           start=False, stop=True)
                EqT = lp.tile([128, 2, 512], BF16, tag="EqT")
                act(EqT[:], qps[:], AFT.Exp, scale=1.0 / SW)
                Sq = lp.tile([128, 2, 8], F32, tag="Sq")
                nc.vector.tensor_reduce(
                    Sq[:], EqT[:].rearrange("p i (h x) -> p i h x", x=64),
                    Ax.X, AluOpType.add)
                rq = lp.tile([128, 2, 8], F32, tag="rq")
                nc.vector.reciprocal(rq[:], Sq[:])
                rq2 = lp.tile([128, 2, 8], F32, tag="rq2")
                ts(rq2[:], rq[:], SQ, None, AluOpType.mult)
                qsmT = lp.tile([128, 2, 512], F8, tag="qsmT")
                tt(qsmT[:].rearrange("p i (h x) -> p i h x", x=64),
                   EqT[:].rearrange("p i (h x) -> p i h x", x=64),
                   rq2[:].unsqueeze(3).broadcast_to([128, 2, 8, 64]),
                   AluOpType.mult)

                # k fp8 DoubleRow (values = SW * k_true)
                kps = psw.tile([128, 2, 512], F32, tag="pw", name="kps")
                for i in range(2):
                    ls = slice(i * 128, (i + 1) * 128)
                    mm(kps[:, i, :], z2f8[:, 0:2, ls], Wkv8[:, 0:2, 0:512],
                       start=True, stop=False, perf_mode=DR)
                    mm(kps[:, i, :], z2f8[:, 2:4, ls], Wkv8[:, 2:4, 0:512],
                       start=False, stop=not HAS_BK, perf_mode=DR)
                    if HAS_BK:
                        mm(kps[:, i, :], ones1x128[:], bk32[:],
                           start=False, stop=True)
                EkT = lp.tile([128, 2, 512], F8, tag="EkT")
                act(EkT[:], kps[:], AFT.Exp, scale=1.0 / SW)

                # v fp8 DoubleRow
                vps = psw.tile([128, 2, 512], F32, tag="pw", name="vps")
                for i in range(2):
                    ls = slice(i * 128, (i + 1) * 128)
                    mm(vps[:, i, :], z2f8[:, 0:2, ls], Wkv8[:, 0:2, 512:1024],
                       start=True, stop=False, perf_mode=DR)
                    mm(vps[:, i, :], z2f8[:, 2:4, ls], Wkv8[:, 2:4, 512:1024],
                       start=False, stop=True, perf_mode=DR)
                vT = lp.tile([128, 2, 512], F8, tag="vT")
                if HAS_BV:
                    for cc in range(CC):
                        cs = slice(cc * 128, (cc + 1) * 128)
                        ts(vT[:, :, cs], vps[:, :, cs], 1.0 / SW,
                           bv_c[:, cc:cc + 1], AluOpType.mult, AluOpType.add)
                elif p % 2 == 0:
                    ts(vT[:], vps[:], 1.0 / SW, None, AluOpType.mult)
                else:
                    pts(vT[:], vps[:], 1.0 / SW, None, AluOpType.mult)

                # ctx/S accumulation over l
                for pr in range(CC):
                    ks = slice(pr * 128, (pr + 1) * 128)
                    mm(ctxps[:, pr, :], EkT[:, :, ks], vT[:, :, ks],
                       start=(p == 0), stop=(p == NP1 - 1), perf_mode=DR,
                       skip_group_check=True)
                    mm(Sps[:, pr:pr + 1], EkT[:, :, ks], ones_f8[:],
                       start=(p == 0), stop=(p == NP1 - 1), perf_mode=DR,
                       skip_group_check=True)

                if p == 0 and deferred_dmas:
                    for _t, _src in deferred_dmas:
                        _ap = _t[:] if hasattr(_t, "tile") else _t
                        nc.sync.dma_start(_ap, _src)
                    deferred_dmas = []

                # transpose qsmT -> channels-first qsm8 (consumed in phase 2a)
                tps = pst.tile([128, 2, 512], F8, tag="tps")
                for i in range(2):
                    for cc in range(CC):
                        cs = slice(cc * 128, (cc + 1) * 128)
                        nc.tensor.transpose(tps[:, i, cs], qsmT[:, i, cs],
                                            identf8[:])
                act(qsm8[:, :, sl].rearrange("p cc (i x) -> p i cc x", x=128),
                    tps[:].rearrange("p i (cc x) -> p i cc x", x=128), AFT.Copy)

            # finalize: ctx_bd = (ctx / S) * SA block-diagonal bf16, then
            # fold Wr: WrCT8[k, o] = sum_v ctx_bd[k, v] * WrT[v, o]  (fp8)
            rs = lp.tile([128, CC], F32, tag="rs", bufs=1)
            nc.vector.reciprocal(rs[:], Sps[:])
            ctxbd = lp.tile([128, CC, 128], BF16, tag="ctxbd", bufs=1)
            ctxbdT = lp.tile([128, CC, 128], BF16, tag="ctxbdT", bufs=1)
            nc.vector.memset(ctxbd[:], 0.0)
            for pr in range(CC):
                for h2 in range(2):
                    s = slice(h2 * 64, (h2 + 1) * 64)
                    ts(ctxbd[s, pr, s], ctxps[s, pr, s], rs[s, pr:pr + 1], SA,
                       AluOpType.mult, AluOpType.mult)
            # (bv, if present, was already folded into v at the vT eviction)
            tpsT = pst.tile([128, CC, 128], BF16, tag="tpsT", name="tpsT")
            for pr in range(CC):
                nc.tensor.transpose(tpsT[:, pr, :], ctxbd[:, pr, :], identb[:])
            nc.vector.tensor_copy(ctxbdT[:], tpsT[:])
            for half in range(2):
                wps = psw.tile([128, 2, 512], F32, tag="pw", name="wps")
                for i in range(2):
                    pr = half * 2 + i
                    mm(wps[:, i, :], ctxbdT[:, pr, :], WrTb[:, pr, :],
                       start=True, stop=True)
                ts(WrCT8[:, half * 2:half * 2 + 2, :], wps[:], 1.0, None,
                   AluOpType.mult)

        # ------------- Phase 2a: fused reprojection + LN1 -> zr --------------
        with ExitStack() as p2a:
            lpa = p2a.enter_context(tc.tile_pool(name="lpa", bufs=3))
            psb = p2a.enter_context(tc.tile_pool(name="psb", bufs=2, space="PSUM"))
            psr = p2a.enter_context(tc.tile_pool(name="psr", bufs=3, space="PSUM"))

            st = [dict() for _ in range(NL2)]

            def a_front(t):
                s = st[t]
                sl = slice(t * 512, (t + 1) * 512)
                z1res = lpa.tile([128, CC, 512], F32, tag="z1res", name="z1res")
                nc.scalar.dma_start(z1res[:], z1r[:, :, sl])
                z = lpa.tile([128, CC, 512], BF16, tag="z", name="z")
                for half in range(2):
                    zps = psb.tile([128, 2, 512], F32, tag="big", name="zps")
                    for i in range(2):
                        ot = half * 2 + i
                        os_ = slice(ot * 128, (ot + 1) * 128)
                        mm(zps[:, i, :], WrCT8[:, 0:2, os_], qsm8[:, 0:2, sl],
                           start=True, stop=False, perf_mode=DR)
                        mm(zps[:, i, :], WrCT8[:, 2:4, os_], qsm8[:, 2:4, sl],
                           start=False, stop=True, perf_mode=DR)
                    hs = slice(half * 2, half * 2 + 2)
                    if half == 0:
                        stt(z[:, hs, :], zps[:], ZDESC, z1res[:, hs, :],
                            AluOpType.mult, AluOpType.add)
                    else:
                        pstt(z[:, hs, :], zps[:], ZDESC, z1res[:, hs, :],
                             AluOpType.mult, AluOpType.add)
                    if HAS_BR:
                        for i in range(2):
                            cc = half * 2 + i
                            ts(z[:, cc, :], z[:, cc, :], br_c[:, cc:cc + 1],
                               None, AluOpType.add)
                s["z"] = z

            def a_mid(t):
                s = st[t]
                z = s["z"]
                zsq = lpa.tile([128, CC, 512], BF16, tag="zsq", name="zsq")
                act(zsq[:], z[:], AFT.Square)
                mups = psr.tile([1, 512], F32, tag="row", name="mups")
                for cc in range(CC):
                    mm(mups[:], inv512b[:], z[:, cc, :], start=(cc == 0),
                       stop=(cc == CC - 1))
                e2ps = psr.tile([1, 512], F32, tag="row", name="e2ps")
                for cc in range(CC):
                    mm(e2ps[:], inv512b[:], zsq[:, cc, :], start=(cc == 0),
                       stop=(cc == CC - 1))
                musq = lpa.tile([1, 512], F32, tag="musq", bufs=2, name="musq")
                ptt(musq[:], mups[:], mups[:], AluOpType.mult)
                varrow = lpa.tile([1, 512], F32, tag="varrow", bufs=2,
                                  name="varrow")
                tt(varrow[:], e2ps[:], musq[:], AluOpType.subtract)
                sig = lpa.tile([1, 512], F32, tag="sig", bufs=2, name="sig")
                act(sig[:], varrow[:], AFT.Sqrt, bias=eps11[0:1, :])
                rrow = lpa.tile([1, 512], F32, tag="rrow", bufs=2, name="rrow")
                nc.vector.reciprocal(rrow[:], sig[:])
                pstt(mur_all[0:1, 0, t, :], mups[:], 8.0, rrow[:],
                     AluOpType.mult, AluOpType.mult)
                s["rrow"] = rrow
                s["mups"] = mups

            def a_back(t):
                s = st[t]
                rbc = psr.tile([128, 512], F32, tag="row", name="rbc")
                mm(rbc[:], ones1x128[:], s["rrow"][:].bitcast(F32R),
                   start=True, stop=True)
                rbcb = lpa.tile([128, 512], BF16, tag="rbcb", name="rbcb")
                act(rbcb[:], rbc[:], AFT.Copy)
                tt(zr_all[:, t, :, :], s["z"][:],
                   rbcb[:].unsqueeze(1).broadcast_to([128, CC, 512]),
                   AluOpType.mult)
                st[t] = {}

            for t in range(NL2 + 2):
                if t < NL2:
                    a_front(t)
                if 1 <= t <= NL2:
                    a_mid(t - 1)
                if 2 <= t <= NL2 + 1:
                    a_back(t - 2)

        # ------------- Phase 2b: FFN1 + ELU + FFN2 + LN2 -> out --------------
        with ExitStack() as p2b:
            lpb = p2b.enter_context(tc.tile_pool(name="lpb", bufs=2))
            lph = p2b.enter_context(tc.tile_pool(name="lph", bufs=1))
            psF = p2b.enter_context(tc.tile_pool(name="psF", bufs=1, space="PSUM"))
            psf = p2b.enter_context(tc.tile_pool(name="psf", bufs=2, space="PSUM"))
            psr2 = p2b.enter_context(tc.tile_pool(name="psr2", bufs=2, space="PSUM"))

            sb = [dict() for _ in range(NL2)]

            def b_post(t, step):
                """LN2 pieces of tile t, emitted interleaved with tile t+1."""
                s = sb[t]
                if step == 0:
                    y = lpb.tile([128, CC, 512], BF16, tag="y", name="y")
                    for cc in range(CC):
                        if cc % 2 == 0:
                            pts(y[:, cc, :], s["f2ps"][:, cc, :],
                                nw2s_c[:, cc:cc + 1], None, AluOpType.add)
                        else:
                            act(y[:, cc, :], s["f2ps"][:, cc, :], AFT.Identity,
                                bias=nw2s_c[:, cc:cc + 1])
                    s["y"] = y
                elif step == 1:
                    ysq = lpb.tile([128, CC, 512], BF16, tag="ysq", name="ysq")
                    tt(ysq[:], s["y"][:], s["y"][:], AluOpType.mult)
                    muy = psr2.tile([1, 512], F32, tag="row", name="muy")
                    for cc in range(CC):
                        mm(muy[:], inv512b[:], s["y"][:, cc, :],
                           start=(cc == 0), stop=(cc == CC - 1))
                    s["ysq"], s["muy"] = ysq, muy
                elif step == 2:
                    e2y = psr2.tile([1, 512], F32, tag="row", name="e2y")
                    for cc in range(CC):
                        mm(e2y[:], inv512b[:], s["ysq"][:, cc, :],
                           start=(cc == 0), stop=(cc == CC - 1))
                    s["e2y"] = e2y
                elif step == 3:
                    mur2 = lpb.tile([1, 512], F32, tag="mur2", bufs=2,
                                    name="mur2")
                    nc.vector.tensor_copy(mur2[:], s["muy"][:])
                    musq2 = lpb.tile([1, 512], F32, tag="musq2", bufs=2,
                                     name="musq2")
                    ptt(musq2[:], mur2[:], mur2[:], AluOpType.mult)
                    s["mur2"], s["musq2"] = mur2, musq2
                elif step == 4:
                    var2 = lpb.tile([1, 512], F32, tag="var2", bufs=2,
                                    name="var2")
                    tt(var2[:], s["e2y"][:], s["musq2"][:], AluOpType.subtract)
                    sig2 = lpb.tile([1, 512], F32, tag="sig2", bufs=2,
                                    name="sig2")
                    act(sig2[:], var2[:], AFT.Sqrt, bias=eps11[0:1, :])
                    s["sig2"] = sig2
                elif step == 5:
                    r2row = lpb.tile([1, 512], F32, tag="r2row", bufs=2,
                                     name="r2row")
                    nc.vector.reciprocal(r2row[:], s["sig2"][:])
                    s["r2row"] = r2row
                elif step == 6:
                    r2bc = psr2.tile([128, 512], F32, tag="row", name="r2bc")
                    mm(r2bc[:], ones1x128[:], s["r2row"][:].bitcast(F32R),
                       start=True, stop=True)
                    mu2bc = psr2.tile([128, 512], F32, tag="row", name="mu2bc")
                    mm(mu2bc[:], ones1x128[:], s["mur2"][:].bitcast(F32R),
                       start=True, stop=True)
                    mu2b = lpb.tile([128, 512], BF16, tag="mu2b", name="mu2b")
                    act(mu2b[:], mu2bc[:], AFT.Copy)
                    s["r2bc"], s["mu2b"] = r2bc, mu2b
                elif step == 7:
                    sl = slice(t * 512, (t + 1) * 512)
                    yc = lpb.tile([128, CC, 512], BF16, tag="yc", name="yc")
                    tt(yc[:], s["y"][:],
                       s["mu2b"][:].unsqueeze(1).broadcast_to([128, CC, 512]),
                       AluOpType.subtract)
                    outt = lpb.tile([128, CC, 512], F32, tag="outt",
                                    name="outt")
                    tt(outt[:], yc[:],
                       s["r2bc"][:].unsqueeze(1).broadcast_to([128, CC, 512]),
                       AluOpType.mult)
                    if HAS_G2:
                        for cc in range(CC):
                            ts(outt[:, cc, :], outt[:, cc, :],
                               g2_c[:, cc:cc + 1], None, AluOpType.mult)
                    if HAS_BE2:
                        for cc in range(CC):
                            ts(outt[:, cc, :], outt[:, cc, :],
                               be2_c[:, cc:cc + 1], None, AluOpType.add)
                    nc.sync.dma_start(outr[:, :, sl], outt[:])
                    sb[t] = {}

            for t in range(NL2):
                f2ps = psF.tile([128, CC, 512], F32, tag="f2", name="f2ps")
                sb[t]["f2ps"] = f2ps
                hes = [None] * 8
                for j in range(8):
                    js = slice(j * 128, (j + 1) * 128)
                    fps = psf.tile([128, 512], F32, tag="fps", name="fps")
                    for cc in range(CC):
                        mm(fps[:], W1gb[:, cc, js], zr_all[:, t, cc, :],
                           start=(cc == 0), stop=False)
                    # fps = h + 1 (row0: -u1/8 x 8*mu*r ; row1: (w1bb+1) x 1)
                    mm(fps[:], U1f[:, :, js], mur_all[:, :, t, :],
                       start=False, stop=True, perf_mode=DR)
                    E = lpb.tile([128, 512], BF16, tag="E", name="E")
                    act(E[:], fps[:], AFT.Exp, bias=negone[:, 0:1])
                    he = lph.tile([128, 512], BF16, tag="he", bufs=4, name="he")
                    # he = elu(h)+1 = max(h+1, min(e^h, 1))
                    if j % 2 == 0:
                        stt(he[:], E[:], 1.0, fps[:], AluOpType.min,
                            AluOpType.max)
                    else:
                        pstt(he[:], E[:], 1.0, fps[:], AluOpType.min,
                             AluOpType.max)
                    hes[j] = he
                    if j > 0:
                        jp = j - 1
                        for o2 in range(CC):
                            mm(f2ps[:, o2, :],
                               W2gb[:, jp, o2 * 128:(o2 + 1) * 128],
                               hes[jp][:], start=(jp == 0), stop=False,
                               skip_group_check=True)
                    if t > 0:
                        b_post(t - 1, j)
                for o2 in range(CC):
                    mm(f2ps[:, o2, :], W2gb[:, 7, o2 * 128:(o2 + 1) * 128],
                       hes[7][:], start=False, stop=True,
                       skip_group_check=True)
            for step in range(8):
                b_post(NL2 - 1, step)

    nc.compile()
    return nc


def _prep_consts(Wq, bq, Wk, bk, Wv, bv, Wr, br, g1, be1, W1, b1, W2, b2, g2, be2):
    import ml_dtypes
    f = np.float32
    f8 = ml_dtypes.float8_e4m3
    bf = ml_dtypes.bfloat16

    def chunkT(a, n):          # [n*128, m] -> [128, n, m]
        return np.ascontiguousarray(a.reshape(n, 128, -1).transpose(1, 0, 2))

    def colsT(v, n):           # [n*128] -> [128, n]
        return np.ascontiguousarray(v.reshape(n, 128).T)

    WqT = np.ascontiguousarray(Wq.T, dtype=f)                       # [c, o]
    WkvT = np.concatenate([Wk.T, Wv.T], axis=1).astype(f)           # [c, k|v]
    WrT = np.ascontiguousarray(Wr.T, dtype=f)                       # [v, o]
    g2_is_one = bool(np.all(g2 == 1.0))
    W1g = (W1 * g1[None, :]).astype(f)                              # [1024, c]
    W2u = (W2 * g2[:, None]).astype(f) if g2_is_one else W2.astype(f)
    u1 = W1g.sum(axis=1).astype(f)
    w1bb = (W1 @ be1 + b1).astype(f)
    w2s = W2u.sum(axis=1).astype(f)                                 # rowsum(W2)

    gates = (
        bool(np.any(bq != 0)), bool(np.any(bk != 0)), bool(np.any(bv != 0)),
        bool(np.any(br != 0)),
        not g2_is_one, bool(np.any(b2 != 0)), bool(np.any(be2 != 0)),
    )
    # gates order used by build: BQ, BK, BV, BR, B2?? -> include b2 into nw2s
    gates = (gates[0], gates[1], gates[2], gates[3], gates[5], gates[4],
             gates[6])
    # nw2s absorbs -rowsum(W2) and +b2
    nw2s = (b2 - w2s).astype(f)

    consts = {
        "Wq8": chunkT(WqT * SW, CC).astype(f8),
        "Wkv8": chunkT(WkvT * SW, CC).astype(f8),
        "WrTb": chunkT(WrT, CC).astype(bf),
        "W1gb": chunkT(np.ascontiguousarray(W1g.T), CC).astype(bf),
        "W2gb": chunkT(np.ascontiguousarray(W2u.T), 8).astype(bf),
        "U1f": np.stack([-u1 / 8.0, w1bb + 1.0])[None].astype(f8),
        "onesrow": np.stack([np.zeros((NL2, 512), np.float32),
                             np.ones((NL2, 512), np.float32)])[None].astype(f8),
        "nw2s_c": colsT(nw2s, CC),
        "inv512b": np.full((128, 1), 1.0 / 512.0, dtype=bf),
        "ones1x128": np.ones((1, 128), dtype=f),
        "identf8": np.eye(128, dtype=f8),
        "identb": np.eye(128, dtype=bf),
        "ones_f8": np.ones((128, 2, 1), dtype=f8),
        "eps11": np.full((1, 1), EPS, dtype=f),
        "negone": np.full((128, 1), -1.0, dtype=f),
        "bq32": (bq * SW).reshape(1, 512).astype(f),
        "bk32": (bk * SW).reshape(1, 512).astype(f),
        "bv_c": colsT(bv.astype(f), CC),
        "br_c": colsT(br.astype(f), CC),
        "g2_c": colsT(g2.astype(f), CC),
        "be2_c": colsT(be2.astype(f), CC),
    }
    return consts, gates


def kernel(**inputs):
    global LAST_RESULT
    z1 = np.asarray(inputs["z1"], dtype=np.float32)
    z2 = np.asarray(inputs["z2"], dtype=np.float32)
    consts, gates = _prep_consts(
        *[np.asarray(inputs[k], dtype=np.float32) for k in
          ["Wq", "bq", "Wk", "bk", "Wv", "bv", "Wr", "br", "g1", "be1",
           "W1", "b1", "W2", "b2", "g2", "be2"]])

    key = ("prog", gates)
    if key not in _CACHE:
        _CACHE.clear()
        _CACHE[key] = _build_program(gates)
    nc = _CACHE[key]

    in_maps = []
    for b in range(B):
        m = dict(consts)
        m["z1"] = np.ascontiguousarray(z1[b])
        m["z2"] = np.ascontiguousarray(z2[b])
        in_maps.append(m)

    import os
    trace = bool(int(os.environ.get("KERNEL_TRACE", "0")))
    res = run_bass_kernel_spmd(nc, in_maps, list(range(B)), trace=trace)
    LAST_RESULT = res
    out = np.stack([res.results[b]["out"] for b in range(B)], axis=0)
    return out.astype(np.float32)


# revision 15
# speedup vs baseline: 1.0474x; 1.0474x over previous
"""CACombiner Trainium2 kernel: conv-projected efficient attention + FFN.

Data-parallel over batch: 8 batch elements -> 8 NeuronCores, identical SPMD
program per core.

Key tricks:
  - q/k/v projections as fp8e4m3 DoubleRow matmuls (K=256/instr).
  - Wr folded into the normalized context: WrCT = ctx_bd^T @ Wr^T computed
    once on-device, so reprojection is a single fp8 DoubleRow GEMM of the
    softmaxed q (stored fp8 x64) -- no att intermediate at all.
  - ELU via elu(x)+1 = max(x+1, min(e^x, 1)): the +1 rides the FFN1 bias
    fold, and the resulting he+1 offset is corrected by subtracting
    rowsum(W2) at the FFN2 eviction.
  - FFN in bf16 (fp8 fails the accuracy budget there); LN stats via
    ones-vector matmuls on bf16 copies.

Structure per core:
  phase 1  (16 x 256-l pairs): q softmax -> qsm8, exp(k), v, ctx/S accum
  phase 2a (8 x 512-l tiles):  fused reprojection + residual, LN1 -> zr
  phase 2b (8 x 512-l tiles):  FFN1 + ELU + FFN2, LN2 -> out
"""
import sys
sys.path.insert(0, "/opt/trn_rl_repo")
from contextlib import ExitStack

import numpy as np

import concourse.bass as bass
import concourse.tile as tile
from concourse import mybir, bacc
from concourse.bass_utils import run_bass_kernel_spmd
from concourse.alu_op_type import AluOpType

F32 = mybir.dt.float32
F32R = mybir.dt.float32r
BF16 = mybir.dt.bfloat16
F8 = mybir.dt.float8e4
AFT = mybir.ActivationFunctionType
Ax = mybir.AxisListType
DR = mybir.MatmulPerfMode.DoubleRow

B, C, L = 8, 512, 4096
H, DK = 8, 64
EPS = 1e-5
CC = C // 128           # 4 channel chunks
NP1 = L // 256          # 16 phase-1 pair-tiles (2x128 l)
NL2 = L // 512          # 8 phase-2 l-tiles

SW = 32.0               # weight scale for fp8
SA = 256.0              # ctx scale for bf16/fp8
SQ = 64.0               # qsm scale for fp8
ZDESC = 1.0 / (SA * SQ)  # descale for fused reprojection output
LN64 = float(np.log(64.0))

_CACHE = {}
LAST_RESULT = None


def _build_program(gates):
    (HAS_BQ, HAS_BK, HAS_BV, HAS_BR, HAS_B2, HAS_G2, HAS_BE2) = gates
    nc = bacc.Bacc("TRN2", target_bir_lowering=False, debug=False)

    def din(name, shape, dtype):
        return nc.dram_tensor(name, list(shape), dtype, kind="ExternalInput").ap()

    z1d = din("z1", (C, L), F32)
    z2d = din("z2", (C, L), F32)
    Wq8_d = din("Wq8", (128, CC, 512), F8)
    Wkv8_d = din("Wkv8", (128, CC, 1024), F8)
    WrTb_d = din("WrTb", (128, CC, 512), BF16)
    W1gb_d = din("W1gb", (128, CC, 1024), BF16)
    W2gb_d = din("W2gb", (128, 8, 512), BF16)
    U1f_d = din("U1f", (1, 2, 1024), F8)
    onesrow_d = din("onesrow", (1, 2, NL2, 512), F8)
    nw2s_c_d = din("nw2s_c", (128, CC), F32)
    inv512b_d = din("inv512b", (128, 1), BF16)
    ones1x128_d = din("ones1x128", (1, 128), F32R)
    identf8_d = din("identf8", (128, 128), F8)
    identb_d = din("identb", (128, 128), BF16)
    ones_f8_d = din("ones_f8", (128, 2, 1), F8)
    eps11_d = din("eps11", (1, 1), F32)
    negone_d = din("negone", (128, 1), F32)
    # gated bias constants (all-zero in the common case)
    bq32_d = din("bq32", (1, 512), F32R)
    bk32_d = din("bk32", (1, 512), F32R)
    bv_c_d = din("bv_c", (128, CC), F32)
    br_c_d = din("br_c", (128, CC), F32)
    g2_c_d = din("g2_c", (128, CC), F32)
    be2_c_d = din("be2_c", (128, CC), F32)
    outd = nc.dram_tensor("out", [C, L], F32, kind="ExternalOutput").ap()

    z1r = z1d.rearrange("(cc p) l -> p cc l", p=128)
    z2r = z2d.rearrange("(cc p) l -> p cc l", p=128)
    outr = outd.rearrange("(cc p) l -> p cc l", p=128)

    mm = nc.tensor.matmul
    tt = nc.vector.tensor_tensor
    ts = nc.vector.tensor_scalar
    stt = nc.vector.scalar_tensor_tensor
    act = nc.scalar.activation
    pts = nc.gpsimd.tensor_scalar
    pstt = nc.gpsimd.scalar_tensor_tensor
    ptt = nc.gpsimd.tensor_tensor
    pcopy = nc.gpsimd.tensor_copy

    with tile.TileContext(nc) as tc, ExitStack() as ctx:
        cpool = ctx.enter_context(tc.tile_pool(name="consts", bufs=1))

        def const_tile(shape, dtype, src, tag, defer=False):
            t = cpool.tile(list(shape), dtype, tag=tag, name=tag)
            if defer:
                deferred_dmas.append((t, src))
            else:
                nc.scalar.dma_start(t[:], src)
            return t

        deferred_dmas = []
        # loaded up front: everything phase 1 touches
        identf8 = const_tile((128, 128), F8, identf8_d, "identf8")
        ones_f8 = const_tile((128, 2, 1), F8, ones_f8_d, "ones_f8")
        Wq8 = const_tile((128, CC, 512), F8, Wq8_d, "Wq8")
        Wkv8 = const_tile((128, CC, 1024), F8, Wkv8_d, "Wkv8")
        inv512b = const_tile((128, 1), BF16, inv512b_d, "inv512b")
        ones1x128 = const_tile((1, 128), F32R, ones1x128_d, "ones1x128")
        identb = const_tile((128, 128), BF16, identb_d, "identb")
        eps11 = const_tile((1, 1), F32, eps11_d, "eps11")
        negone = const_tile((128, 1), F32, negone_d, "negone")
        # loaded during phase 1 (consumed by finalize / phase 2)
        WrTb = const_tile((128, CC, 512), BF16, WrTb_d, "WrTb", defer=True)
        W1gb = const_tile((128, CC, 1024), BF16, W1gb_d, "W1gb", defer=True)
        W2gb = const_tile((128, 8, 512), BF16, W2gb_d, "W2gb", defer=True)
        U1f = const_tile((1, 2, 1024), F8, U1f_d, "U1f", defer=True)
        nw2s_c = const_tile((128, CC), F32, nw2s_c_d, "nw2s_c", defer=True)
        if HAS_BQ:
            bq32 = const_tile((1, 512), F32R, bq32_d, "bq32")
        if HAS_BK:
            bk32 = const_tile((1, 512), F32R, bk32_d, "bk32")
        if HAS_BV:
            bv_c = const_tile((128, CC), F32, bv_c_d, "bv_c")
        if HAS_BR:
            br_c = const_tile((128, CC), F32, br_c_d, "br_c")
        if HAS_G2:
            g2_c = const_tile((128, CC), F32, g2_c_d, "g2_c")
        if HAS_BE2:
            be2_c = const_tile((128, CC), F32, be2_c_d, "be2_c")

        # persistent across phases
        qsm8 = cpool.tile([128, CC, L], F8, tag="qsm8", name="qsm8")
        WrCT8 = cpool.tile([128, CC, 512], F8, tag="WrCT8", name="WrCT8")
        zr_all = cpool.tile([128, NL2, CC, 512], BF16, tag="zr", name="zr_all")
        mur_all = cpool.tile([1, 2, NL2, 512], F8, tag="mur", name="mur_all")
        deferred_dmas.append((mur_all[:], onesrow_d))

        # ---------------- Phase 1: q softmax + k/v + ctx accumulation --------
        with ExitStack() as p1:
            lp = p1.enter_context(tc.tile_pool(name="lp1", bufs=2))
            psw = p1.enter_context(tc.tile_pool(name="psw", bufs=2, space="PSUM"))
            pst = p1.enter_context(tc.tile_pool(name="pst", bufs=1, space="PSUM"))
            psc = p1.enter_context(tc.tile_pool(name="psc", bufs=1, space="PSUM"))

            ctxps = psc.tile([128, CC, 128], F32, tag="ctxps", name="ctxps")
            Sps = psc.tile([128, CC], F32, tag="Sps", name="Sps")

            for p in range(NP1):
                l0 = p * 256
                sl = slice(l0, l0 + 256)
                z1c = lp.tile([128, CC, 256], F32, tag="z1c")
                nc.sync.dma_start(z1c[:], z1r[:, :, sl])
                z2c = lp.tile([128, CC, 256], F32, tag="z2c")
                nc.sync.dma_start(z2c[:], z2r[:, :, sl])
                z1f8 = lp.tile([128, CC, 256], F8, tag="z1f8")
                pcopy(z1f8[:], z1c[:])
                z2f8 = lp.tile([128, CC, 256], F8, tag="z2f8")
                pcopy(z2f8[:], z2c[:])

                # qT [l,o] fp8 DoubleRow (values = SW * q_true)
                qps = psw.tile([128, 2, 512], F32, tag="pw", name="qps")
                for i in range(2):
                    ls = slice(i * 128, (i + 1) * 128)
                    mm(qps[:, i, :], z1f8[:, 0:2, ls], Wq8[:, 0:2, :],
                       start=True, stop=False, perf_mode=DR)
                    mm(qps[:, i, :], z1f8[:, 2:4, ls], Wq8[:, 2:4, :],
                       start=False, stop=not HAS_BQ, perf_mode=DR)
                    if HAS_BQ:
                        mm(qps[:, i, :], ones1x128[:], bq32[:],
                           start=False, stop=True)
                EqT = lp.tile([128, 2, 512], BF16, tag="EqT")
                act(EqT[:], qps[:], AFT.Exp, scale=1.0 / SW)
                Sq = lp.tile([128, 2, 8], F32, tag="Sq")
                nc.vector.tensor_reduce(
                    Sq[:], EqT[:].rearrange("p i (h x) -> p i h x", x=64),
                    Ax.X, AluOpType.add)
                rq = lp.tile([128, 2, 8], F32, tag="rq")
                nc.vector.reciprocal(rq[:], Sq[:])
                rq2 = lp.tile([128, 2, 8], F32, tag="rq2")
                ts(rq2[:], rq[:], SQ, None, AluOpType.mult)
                qsmT = lp.tile([128, 2, 512], F8, tag="qsmT")
                tt(qsmT[:].rearrange("p i (h x) -> p i h x", x=64),
                   EqT[:].rearrange("p i (h x) -> p i h x", x=64),
                   rq2[:].unsqueeze(3).broadcast_to([128, 2, 8, 64]),
                   AluOpType.mult)

                # k fp8 DoubleRow (values = SW * k_true)
                kps = psw.tile([128, 2, 512], F32, tag="pw", name="kps")
                for i in range(2):
                    ls = slice(i * 128, (i + 1) * 128)
                    mm(kps[:, i, :], z2f8[:, 0:2, ls], Wkv8[:, 0:2, 0:512],
                       start=True, stop=False, perf_mode=DR)
                    mm(kps[:, i, :], z2f8[:, 2:4, ls], Wkv8[:, 2:4, 0:512],
                       start=False, stop=not HAS_BK, perf_mode=DR)
                    if HAS_BK:
                        mm(kps[:, i, :], ones1x128[:], bk32[:],
                           start=False, stop=True)
                EkT = lp.tile([128, 2, 512], F8, tag="EkT")
                act(EkT[:], kps[:], AFT.Exp, scale=1.0 / SW)

                # v fp8 DoubleRow
                vps = psw.tile([128, 2, 512], F32, tag="pw", name="vps")
                for i in range(2):
                    ls = slice(i * 128, (i + 1) * 128)
                    mm(vps[:, i, :], z2f8[:, 0:2, ls], Wkv8[:, 0:2, 512:1024],
                       start=True, stop=False, perf_mode=DR)
                    mm(vps[:, i, :], z2f8[:, 2:4, ls], Wkv8[:, 2:4, 512:1024],
                       start=False, stop=True, perf_mode=DR)
                vT = lp.tile([128, 2, 512], F8, tag="vT")
                if HAS_BV:
                    for cc in range(CC):
                        cs = slice(cc * 128, (cc + 1) * 128)
                        ts(vT[:, :, cs], vps[:, :, cs], 1.0 / SW,
                           bv_c[:, cc:cc + 1], AluOpType.mult, AluOpType.add)
                elif p % 2 == 0:
                    ts(vT[:], vps[:], 1.0 / SW, None, AluOpType.mult)
                else:
                    pts(vT[:], vps[:], 1.0 / SW, None, AluOpType.mult)

                # ctx/S accumulation over l
                for pr in range(CC):
                    ks = slice(pr * 128, (pr + 1) * 128)
                    mm(ctxps[:, pr, :], EkT[:, :, ks], vT[:, :, ks],
                       start=(p == 0), stop=(p == NP1 - 1), perf_mode=DR,
                       skip_group_check=True)
                    mm(Sps[:, pr:pr + 1], EkT[:, :, ks], ones_f8[:],
                       start=(p == 0), stop=(p == NP1 - 1), perf_mode=DR,
                       skip_group_check=True)

                if p == 0 and deferred_dmas:
                    for _t, _src in deferred_dmas:
                        _ap = _t[:] if hasattr(_t, "tile") else _t
                        nc.sync.dma_start(_ap, _src)
                    deferred_dmas = []

                # transpose qsmT -> channels-first qsm8 (consumed in phase 2a)
                tps = pst.tile([128, 2, 512], F8, tag="tps")
                for i in range(2):
                    for cc in range(CC):
                        cs = slice(cc * 128, (cc + 1) * 128)
                        nc.tensor.transpose(tps[:, i, cs], qsmT[:, i, cs],
                                            identf8[:])
                act(qsm8[:, :, sl].rearrange("p cc (i x) -> p i cc x", x=128),
                    tps[:].rearrange("p i (cc x) -> p i cc x", x=128), AFT.Copy)

            # finalize: ctx_bd = (ctx / S) * SA block-diagonal bf16, then
            # fold Wr: WrCT8[k, o] = sum_v ctx_bd[k, v] * WrT[v, o]  (fp8)
            rs = lp.tile([128, CC], F32, tag="rs", bufs=1)
            nc.vector.reciprocal(rs[:], Sps[:])
            ctxbd = lp.tile([128, CC, 128], BF16, tag="ctxbd", bufs=1)
            ctxbdT = lp.tile([128, CC, 128], BF16, tag="ctxbdT", bufs=1)
            nc.vector.memset(ctxbd[:], 0.0)
            for pr in range(CC):
                for h2 in range(2):
                    s = slice(h2 * 64, (h2 + 1) * 64)
                    ts(ctxbd[s, pr, s], ctxps[s, pr, s], rs[s, pr:pr + 1], SA,
                       AluOpType.mult, AluOpType.mult)
            # (bv, if present, was already folded into v at the vT eviction)
            tpsT = pst.tile([128, CC, 128], BF16, tag="tpsT", name="tpsT")
            for pr in range(CC):
                nc.tensor.transpose(tpsT[:, pr, :], ctxbd[:, pr, :], identb[:])
            nc.vector.tensor_copy(ctxbdT[:], tpsT[:])
            for half in range(2):
                wps = psw.tile([128, 2, 512], F32, tag="pw", name="wps")
                for i in range(2):
                    pr = half * 2 + i
                    mm(wps[:, i, :], ctxbdT[:, pr, :], WrTb[:, pr, :],
                       start=True, stop=True)
                ts(WrCT8[:, half * 2:half * 2 + 2, :], wps[:], 1.0, None,
                   AluOpType.mult)

        # ------------- Phase 2a: fused reprojection + LN1 -> zr --------------
        with ExitStack() as p2a:
            lpa = p2a.enter_context(tc.tile_pool(name="lpa", bufs=3))
            psb = p2a.enter_context(tc.tile_pool(name="psb", bufs=2, space="PSUM"))
            psr = p2a.enter_context(tc.tile_pool(name="psr", bufs=3, space="PSUM"))

            st = [dict() for _ in range(NL2)]

            def a_front(t):
                s = st[t]
                sl = slice(t * 512, (t + 1) * 512)
                z1res = lpa.tile([128, CC, 512], F32, tag="z1res", name="z1res")
                nc.scalar.dma_start(z1res[:], z1r[:, :, sl])
                z = lpa.tile([128, CC, 512], BF16, tag="z", name="z")
                for half in range(2):
                    zps = psb.tile([128, 2, 512], F32, tag="big", name="zps")
                    for i in range(2):
                        ot = half * 2 + i
                        os_ = slice(ot * 128, (ot + 1) * 128)
                        mm(zps[:, i, :], WrCT8[:, 0:2, os_], qsm8[:, 0:2, sl],
                           start=True, stop=False, perf_mode=DR)
                        mm(zps[:, i, :], WrCT8[:, 2:4, os_], qsm8[:, 2:4, sl],
                           start=False, stop=True, perf_mode=DR)
                    hs = slice(half * 2, half * 2 + 2)
                    if half == 0:
                        stt(z[:, hs, :], zps[:], ZDESC, z1res[:, hs, :],
                            AluOpType.mult, AluOpType.add)
                    else:
                        pstt(z[:, hs, :], zps[:], ZDESC, z1res[:, hs, :],
                             AluOpType.mult, AluOpType.add)
                    if HAS_BR:
                        for i in range(2):
                            cc = half * 2 + i
                            ts(z[:, cc, :], z[:, cc, :], br_c[:, cc:cc + 1],
                               None, AluOpType.add)
                s["z"] = z

            def a_mid(t):
                s = st[t]
                z = s["z"]
                zsq = lpa.tile([128, CC, 512], BF16, tag="zsq", name="zsq")
                tt(zsq[:], z[:], z[:], AluOpType.mult)
                mups = psr.tile([1, 512], F32, tag="row", name="mups")
                for cc in range(CC):
                    mm(mups[:], inv512b[:], z[:, cc, :], start=(cc == 0),
                       stop=(cc == CC - 1))
                e2ps = psr.tile([1, 512], F32, tag="row", name="e2ps")
                for cc in range(CC):
                    mm(e2ps[:], inv512b[:], zsq[:, cc, :], start=(cc == 0),
                       stop=(cc == CC - 1))
                musq = lpa.tile([1, 512], F32, tag="musq", bufs=2, name="musq")
                ptt(musq[:], mups[:], mups[:], AluOpType.mult)
                varrow = lpa.tile([1, 512], F32, tag="varrow", bufs=2,
                                  name="varrow")
                tt(varrow[:], e2ps[:], musq[:], AluOpType.subtract)
                sig = lpa.tile([1, 512], F32, tag="sig", bufs=2, name="sig")
                act(sig[:], varrow[:], AFT.Sqrt, bias=eps11[0:1, :])
                rrow = lpa.tile([1, 512], F32, tag="rrow", bufs=2, name="rrow")
                nc.vector.reciprocal(rrow[:], sig[:])
                pstt(mur_all[0:1, 0, t, :], mups[:], 8.0, rrow[:],
                     AluOpType.mult, AluOpType.mult)
                s["rrow"] = rrow
                s["mups"] = mups

            def a_back(t):
                s = st[t]
                rbc = psr.tile([128, 512], F32, tag="row", name="rbc")
                mm(rbc[:], ones1x128[:], s["rrow"][:].bitcast(F32R),
                   start=True, stop=True)
                rbcb = lpa.tile([128, 512], BF16, tag="rbcb", name="rbcb")
                act(rbcb[:], rbc[:], AFT.Copy)
                tt(zr_all[:, t, :, :], s["z"][:],
                   rbcb[:].unsqueeze(1).broadcast_to([128, CC, 512]),
                   AluOpType.mult)
                st[t] = {}

            for t in range(NL2 + 2):
                if t < NL2:
                    a_front(t)
                if 1 <= t <= NL2:
                    a_mid(t - 1)
                if 2 <= t <= NL2 + 1:
                    a_back(t - 2)

        # ------------- Phase 2b: FFN1 + ELU + FFN2 + LN2 -> out --------------
        with ExitStack() as p2b:
            lpb = p2b.enter_context(tc.tile_pool(name="lpb", bufs=2))
            lph = p2b.enter_context(tc.tile_pool(name="lph", bufs=1))
            psF = p2b.enter_context(tc.tile_pool(name="psF", bufs=1, space="PSUM"))
            psf = p2b.enter_context(tc.tile_pool(name="psf", bufs=2, space="PSUM"))
            psr2 = p2b.enter_context(tc.tile_pool(name="psr2", bufs=2, space="PSUM"))

            sb = [dict() for _ in range(NL2)]

            def b_post(t, step):
                """LN2 pieces of tile t, emitted interleaved with tile t+1."""
                s = sb[t]
                if step == 0:
                    y = lpb.tile([128, CC, 512], BF16, tag="y", name="y")
                    for cc in range(CC):
                        if cc % 2 == 0:
                            pts(y[:, cc, :], s["f2ps"][:, cc, :],
                                nw2s_c[:, cc:cc + 1], None, AluOpType.add)
                        else:
                            act(y[:, cc, :], s["f2ps"][:, cc, :], AFT.Identity,
                                bias=nw2s_c[:, cc:cc + 1])
                    s["y"] = y
                elif step == 1:
                    ysq = lpb.tile([128, CC, 512], BF16, tag="ysq", name="ysq")
                    tt(ysq[:], s["y"][:], s["y"][:], AluOpType.mult)
                    muy = psr2.tile([1, 512], F32, tag="row", name="muy")
                    for cc in range(CC):
                        mm(muy[:], inv512b[:], s["y"][:, cc, :],
                           start=(cc == 0), stop=(cc == CC - 1))
                    s["ysq"], s["muy"] = ysq, muy
                elif step == 2:
                    e2y = psr2.tile([1, 512], F32, tag="row", name="e2y")
                    for cc in range(CC):
                        mm(e2y[:], inv512b[:], s["ysq"][:, cc, :],
                           start=(cc == 0), stop=(cc == CC - 1))
                    s["e2y"] = e2y
                elif step == 3:
                    mur2 = lpb.tile([1, 512], F32, tag="mur2", bufs=2,
                                    name="mur2")
                    nc.vector.tensor_copy(mur2[:], s["muy"][:])
                    musq2 = lpb.tile([1, 512], F32, tag="musq2", bufs=2,
                                     name="musq2")
                    ptt(musq2[:], mur2[:], mur2[:], AluOpType.mult)
                    s["mur2"], s["musq2"] = mur2, musq2
                elif step == 4:
                    var2 = lpb.tile([1, 512], F32, tag="var2", bufs=2,
                                    name="var2")
                    tt(var2[:], s["e2y"][:], s["musq2"][:], AluOpType.subtract)
                    sig2 = lpb.tile([1, 512], F32, tag="sig2", bufs=2,
                                    name="sig2")
                    act(sig2[:], var2[:], AFT.Sqrt, bias=eps11[0:1, :])
                    s["sig2"] = sig2
                elif step == 5:
                    r2row = lpb.tile([1, 512], F32, tag="r2row", bufs=2,
                                     name="r2row")
                    nc.vector.reciprocal(r2row[:], s["sig2"][:])
                    s["r2row"] = r2row
                elif step == 6:
                    r2bc = psr2.tile([128, 512], F32, tag="row", name="r2bc")
                    mm(r2bc[:], ones1x128[:], s["r2row"][:].bitcast(F32R),
                       start=True, stop=True)
                    mu2bc = psr2.tile([128, 512], F32, tag="row", name="mu2bc")
                    mm(mu2bc[:], ones1x128[:], s["mur2"][:].bitcast(F32R),
                       start=True, stop=True)
                    mu2b = lpb.tile([128, 512], BF16, tag="mu2b", name="mu2b")
                    act(mu2b[:], mu2bc[:], AFT.Copy)
                    s["r2bc"], s["mu2b"] = r2bc, mu2b
                elif step == 7:
                    sl = slice(t * 512, (t + 1) * 512)
                    yc = lpb.tile([128, CC, 512], BF16, tag="yc", name="yc")
                    tt(yc[:], s["y"][:],
                       s["mu2b"][:].unsqueeze(1).broadcast_to([128, CC, 512]),
                       AluOpType.subtract)
                    outt = lpb.tile([128, CC, 512], F32, tag="outt",
                                    name="outt")
                    tt(outt[:], yc[:],
                       s["r2bc"][:].unsqueeze(1).broadcast_to([128, CC, 512]),
                       AluOpType.mult)
                    if HAS_G2:
                        for cc in range(CC):
                            ts(outt[:, cc, :], outt[:, cc, :],
                               g2_c[:, cc:cc + 1], None, AluOpType.mult)
                    if HAS_BE2:
                        for cc in range(CC):
                            ts(outt[:, cc, :], outt[:, cc, :],
                               be2_c[:, cc:cc + 1], None, AluOpType.add)
                    nc.sync.dma_start(outr[:, :, sl], outt[:])
                    sb[t] = {}

            for t in range(NL2):
                f2ps = psF.tile([128, CC, 512], F32, tag="f2", name="f2ps")
                sb[t]["f2ps"] = f2ps
                hes = [None] * 8
                for j in range(8):
                    js = slice(j * 128, (j + 1) * 128)
                    fps = psf.tile([128, 512], F32, tag="fps", name="fps")
                    for cc in range(CC):
                        mm(fps[:], W1gb[:, cc, js], zr_all[:, t, cc, :],
                           start=(cc == 0), stop=False)
                    # fps = h + 1 (row0: -u1/8 x 8*mu*r ; row1: (w1bb+1) x 1)
                    mm(fps[:], U1f[:, :, js], mur_all[:, :, t, :],
                       start=False, stop=True, perf_mode=DR)
                    E = lpb.tile([128, 512], BF16, tag="E", name="E")
                    act(E[:], fps[:], AFT.Exp, bias=negone[:, 0:1])
                    he = lph.tile([128, 512], BF16, tag="he", bufs=4, name="he")
                    # he = elu(h)+1 = max(h+1, min(e^h, 1))
                    if j % 2 == 0:
                        stt(he[:], E[:], 1.0, fps[:], AluOpType.min,
                            AluOpType.max)
                    else:
                        pstt(he[:], E[:], 1.0, fps[:], AluOpType.min,
                             AluOpType.max)
                    hes[j] = he
                    if j > 1:
                        jp = j - 2
                        for o2 in range(CC):
                            mm(f2ps[:, o2, :],
                               W2gb[:, jp, o2 * 128:(o2 + 1) * 128],
                               hes[jp][:], start=(jp == 0), stop=False,
                               skip_group_check=True)
                    if t > 0:
                        b_post(t - 1, j)
                for jp in (6, 7):
                    for o2 in range(CC):
                        mm(f2ps[:, o2, :], W2gb[:, jp, o2 * 128:(o2 + 1) * 128],
                           hes[jp][:], start=False, stop=(jp == 7),
                           skip_group_check=True)
            for step in range(8):
                b_post(NL2 - 1, step)

    nc.compile()
    return nc


def _prep_consts(Wq, bq, Wk, bk, Wv, bv, Wr, br, g1, be1, W1, b1, W2, b2, g2, be2):
    import ml_dtypes
    f = np.float32
    f8 = ml_dtypes.float8_e4m3
    bf = ml_dtypes.bfloat16

    def chunkT(a, n):          # [n*128, m] -> [128, n, m]
        return np.ascontiguousarray(a.reshape(n, 128, -1).transpose(1, 0, 2))

    def colsT(v, n):           # [n*128] -> [128, n]
        return np.ascontiguousarray(v.reshape(n, 128).T)

    WqT = np.ascontiguousarray(Wq.T, dtype=f)                       # [c, o]
    WkvT = np.concatenate([Wk.T, Wv.T], axis=1).astype(f)           # [c, k|v]
    WrT = np.ascontiguousarray(Wr.T, dtype=f)                       # [v, o]
    g2_is_one = bool(np.all(g2 == 1.0))
    W1g = (W1 * g1[None, :]).astype(f)                              # [1024, c]
    W2u = (W2 * g2[:, None]).astype(f) if g2_is_one else W2.astype(f)
    u1 = W1g.sum(axis=1).astype(f)
    w1bb = (W1 @ be1 + b1).astype(f)
    w2s = W2u.sum(axis=1).astype(f)                                 # rowsum(W2)

    gates = (
        bool(np.any(bq != 0)), bool(np.any(bk != 0)), bool(np.any(bv != 0)),
        bool(np.any(br != 0)),
        not g2_is_one, bool(np.any(b2 != 0)), bool(np.any(be2 != 0)),
    )
    # gates order used by build: BQ, BK, BV, BR, B2?? -> include b2 into nw2s
    gates = (gates[0], gates[1], gates[2], gates[3], gates[5], gates[4],
             gates[6])
    # nw2s absorbs -rowsum(W2) and +b2
    nw2s = (b2 - w2s).astype(f)

    consts = {
        "Wq8": chunkT(WqT * SW, CC).astype(f8),
        "Wkv8": chunkT(WkvT * SW, CC).astype(f8),
        "WrTb": chunkT(WrT, CC).astype(bf),
        "W1gb": chunkT(np.ascontiguousarray(W1g.T), CC).astype(bf),
        "W2gb": chunkT(np.ascontiguousarray(W2u.T), 8).astype(bf),
        "U1f": np.stack([-u1 / 8.0, w1bb + 1.0])[None].astype(f8),
        "onesrow": np.stack([np.zeros((NL2, 512), np.float32),
                             np.ones((NL2, 512), np.float32)])[None].astype(f8),
        "nw2s_c": colsT(nw2s, CC),
        "inv512b": np.full((128, 1), 1.0 / 512.0, dtype=bf),
        "ones1x128": np.ones((1, 128), dtype=f),
        "identf8": np.eye(128, dtype=f8),
        "identb": np.eye(128, dtype=bf),
        "ones_f8": np.ones((128, 2, 1), dtype=f8),
        "eps11": np.full((1, 1), EPS, dtype=f),
        "negone": np.full((128, 1), -1.0, dtype=f),
        "bq32": (bq * SW).reshape(1, 512).astype(f),
        "bk32": (bk * SW).reshape(1, 512).astype(f),
        "bv_c": colsT(bv.astype(f), CC),
        "br_c": colsT(br.astype(f), CC),
        "g2_c": colsT(g2.astype(f), CC),
        "be2_c": colsT(be2.astype(f), CC),
    }
    return consts, gates


def kernel(**inputs):
    global LAST_RESULT
    z1 = np.asarray(inputs["z1"], dtype=np.float32)
    z2 = np.asarray(inputs["z2"], dtype=np.float32)
    consts, gates = _prep_consts(
        *[np.asarray(inputs[k], dtype=np.float32) for k in
          ["Wq", "bq", "Wk", "bk", "Wv", "bv", "Wr", "br", "g1", "be1",
           "W1", "b1", "W2", "b2", "g2", "be2"]])

    key = ("prog", gates)
    if key not in _CACHE:
        _CACHE.clear()
        _CACHE[key] = _build_program(gates)
    nc = _CACHE[key]

    in_maps = []
    for b in range(B):
        m = dict(consts)
        m["z1"] = np.ascontiguousarray(z1[b])
        m["z2"] = np.ascontiguousarray(z2[b])
        in_maps.append(m)

    import os
    trace = bool(int(os.environ.get("KERNEL_TRACE", "0")))
    res = run_bass_kernel_spmd(nc, in_maps, list(range(B)), trace=trace)
    LAST_RESULT = res
    out = np.stack([res.results[b]["out"] for b in range(B)], axis=0)
    return out.astype(np.float32)
